# revision 10
# baseline (speedup 1.0000x reference)
"""Trainium2 Bass kernel for the DiffPool-style GCN forward pass.

Computation (dead softmax/pool branches of the reference are skipped):
    x1 = relu(Dh (A+I) Dh (x @ W1e) + b1e)
    x2 = relu(Dh (A+I) Dh (x1 @ W2e) + b2e)
    out = (graph_mean_pool(x2) @ Wlin) + blin          -> [64, 10] fp32

Key reassociation: aggregation is linear, so
    x_l+1 = relu([dinv_dst * agg(dinv_src * x_l) + dinv_dst^2 * x_l] @ W + b)

v2 structure (vs the 2-phase v1): the layer-2 SWDGE gather pipeline is the
critical resource (~2.3us per 1024-row call on both Q7 desc-gen and SDMA
drain; >1024-row calls crash the ucode). So the table is split into S=3
source segments, each AllGathered as soon as layer 1 finishes its windows,
and the gather calls for segment s run as one continuous GpSimd stream
starting ~40us into the kernel, overlapping the rest of layer 1, the later
AGs, and all PE/DVE work. Windows are 64 dst wide (halves PE matmul cycles
and one-hot build columns); layer-1's edge-aligned x stream is fp8.
"""

import numpy as np
import ml_dtypes

N = 50000
E = 800000
G = 64
C = 128
C_OUT = 10
NCORES = 8
NLOC = N // NCORES          # 6250
WD = 64                     # dst window width
W = (NLOC + WD - 1) // WD   # 98 windows
NPAIR = (W + 1) // 2        # 49 pairs of windows (128 dst each)
NPAD = W * WD               # 6272
NSEG = 3
SEG_PAIRS = [16, 16, 17]    # window-pairs per source segment
SEG_ROWS = [p * 128 for p in SEG_PAIRS[:-1]] + [NLOC - 32 * 128]  # 2048,2048,2154
SEG_START = [0, 2048, 4096]
MAX_CALL_CHUNKS = 8         # 1024 rows per dma_gather call (ucode cap)
XE_SLAB = 32                # edge-aligned x chunks per stream DMA
OH_GROUP = 32               # one-hot chunks per DVE build
NQ = 4                      # SWDGE queues (ucode max)
AG_PREFIX = 10              # seg-0 gather calls issued before the AG1 trigger

BF16 = ml_dtypes.bfloat16
FP8 = ml_dtypes.float8_e4m3fn

_CACHE = {}


def _build_program(plan):
    import concourse.bacc as bacc
    import concourse.mybir as mybir
    import concourse.tile as tile
    from concourse import library_config
    from concourse.bass_interp import get_hw_module
    from concourse.tile_rust import add_dep_helper

    f32 = mybir.dt.float32
    bf16 = mybir.dt.bfloat16
    fp8 = mybir.dt.float8e4
    i16 = mybir.dt.int16
    Relu = mybir.ActivationFunctionType.Relu
    Copy = mybir.ActivationFunctionType.Copy
    Mult = mybir.AluOpType.mult

    f_chunks = plan["f_chunks"]        # [W] layer-1 chunks per window
    win_f_base = plan["win_f_base"]
    TCH = plan["TCH"]
    g_chunks = plan["g_chunks"]        # [NSEG][W]
    win_g_base = plan["win_g_base"]    # [NSEG][W]
    seg_calls = plan["seg_calls"]      # [NSEG] list of (start_chunk, nch, idx_col)
    seg_colbase = plan["seg_colbase"]  # [NSEG] drel col base
    TC = plan["total_drel_cols"]
    TIC = plan["total_idxcols"]

    nc = bacc.Bacc("TRN2", target_bir_lowering=False, debug=False,
                   num_devices=NCORES, num_swdge_queues=NQ)

    # ---- I/O ----
    xe_in = nc.dram_tensor("xedge", [C, TCH * C], fp8, kind="ExternalInput")
    xtl2_in = nc.dram_tensor("xTl2", [C, NPAD], bf16, kind="ExternalInput")
    dvr_in = nc.dram_tensor("dinvrow", [C, NPAD], bf16, kind="ExternalInput")
    idx_in = nc.dram_tensor("idx16", [C, TIC], i16, kind="ExternalInput")
    drel_in = nc.dram_tensor("drelb", [C, TC], bf16, kind="ExternalInput")
    iota_in = nc.dram_tensor("iotab", [C, WD], bf16, kind="ExternalInput")
    sel_in = nc.dram_tensor("selg", [C, NPAIR * G], bf16, kind="ExternalInput")
    ident_in = nc.dram_tensor("identb", [C, C], bf16, kind="ExternalInput")
    w1_in = nc.dram_tensor("w1e", [C, C], bf16, kind="ExternalInput")
    w2_in = nc.dram_tensor("w2e", [C, C], bf16, kind="ExternalInput")
    wlin_in = nc.dram_tensor("wlin", [C, C_OUT], bf16, kind="ExternalInput")
    b1_in = nc.dram_tensor("b1row", [1, C], bf16, kind="ExternalInput")
    b2_in = nc.dram_tensor("b2row", [1, C], bf16, kind="ExternalInput")
    ones_in = nc.dram_tensor("ones1", [1, C], bf16, kind="ExternalInput")
    dinvw_in = nc.dram_tensor("dinvw", [C, NPAIR], f32, kind="ExternalInput")
    dinvw2_in = nc.dram_tensor("dinvw2", [C, NPAIR], f32, kind="ExternalInput")
    blin_in = nc.dram_tensor("blinb", [G, C_OUT], f32, kind="ExternalInput")
    icnt_in = nc.dram_tensor("invcnt", [G, 1], f32, kind="ExternalInput")
    out_t = nc.dram_tensor("out", [G, C_OUT], f32, kind="ExternalOutput")

    with tile.TileContext(nc) as tc:
        with tc.tile_pool(name="res", bufs=1) as res, \
             tc.tile_pool(name="gp", bufs=8) as gp, \
             tc.tile_pool(name="xep", bufs=3) as xep, \
             tc.tile_pool(name="ohp", bufs=3) as ohp, \
             tc.tile_pool(name="st2", bufs=3) as st2p, \
             tc.tile_pool(name="hx", bufs=4) as hxp, \
             tc.tile_pool(name="psw", bufs=2, space="PSUM") as psw, \
             tc.tile_pool(name="psd", bufs=2, space="PSUM") as psd, \
             tc.tile_pool(name="pstr", bufs=1, space="PSUM") as pstr, \
             tc.tile_pool(name="psp", bufs=1, space="PSUM") as psp, \
             tc.tile_pool(name="dram", bufs=1, space="DRAM") as dram:

            lib = nc.gpsimd.load_library(library_config.mlp)

            def load_res(name, src, shape, dt=f32):
                t = res.tile(shape, dt, tag=name)
                nc.sync.dma_start(out=t[:], in_=src[:])
                return t

            idx16 = load_res("r_idx", idx_in, [C, TIC], i16)
            drel = load_res("r_drel", drel_in, [C, TC], bf16)
            iota = load_res("r_iota", iota_in, [C, WD], bf16)
            xTl2 = load_res("r_xtl2", xtl2_in, [C, NPAD], bf16)
            dinvrow = load_res("r_dvr", dvr_in, [C, NPAD], bf16)
            selg = load_res("r_sel", sel_in, [C, NPAIR * G], bf16)
            identb = load_res("r_id", ident_in, [C, C], bf16)
            w1 = load_res("r_w1", w1_in, [C, C], bf16)
            w2 = load_res("r_w2", w2_in, [C, C], bf16)
            wlin = load_res("r_wl", wlin_in, [C, C_OUT], bf16)
            bias1 = load_res("r_b1", b1_in, [1, C], bf16)
            bias2 = load_res("r_b2", b2_in, [1, C], bf16)
            ones1 = load_res("r_on", ones_in, [1, C], bf16)
            dinvw = load_res("r_dw", dinvw_in, [C, NPAIR])
            dinvw2 = load_res("r_dw2", dinvw2_in, [C, NPAIR])
            blinb = load_res("r_bl", blin_in, [G, C_OUT])
            icnt = load_res("r_ic", icnt_in, [G, 1])

            x1T2 = res.tile([C, NPAD], bf16)   # dinv^2 * x1^T
            accT = res.tile([C, NPAD], f32)    # layer-2 raw agg accumulator

            # ---- DRAM buffers ----
            ag_in = [dram.tile([SEG_ROWS[s], C], bf16, name=f"agin{s}")
                     for s in range(NSEG)]
            tables = [dram.tile([SEG_ROWS[s] * NCORES, C], bf16,
                                name=f"table{s}")
                      for s in range(NSEG)]
            ar_in = dram.tile([C, G], f32)
            ar_out = dram.tile([C, G], f32)
            rg = [list(range(NCORES))]

            def allgather(src, dst):
                nc.gpsimd.collective_compute(
                    "AllGather", mybir.AluOpType.bypass, replica_groups=rg,
                    ins=[src.opt()], outs=[dst.opt()])

            # ---- shared one-hot builder over unified drel col space ----
            ohtiles = {}

            def ensure_oh(gidx):
                g0 = (gidx // OH_GROUP) * OH_GROUP
                oht = ohtiles.get(g0)
                if oht is None:
                    take = min(OH_GROUP, TC - g0)
                    oht = ohp.tile([C, OH_GROUP * WD], bf16, tag="oh",
                                   name="ohbuf")
                    dcols = drel[:, g0:g0 + take]
                    nc.vector.tensor_tensor(
                        out=oht[:, 0:take * WD]
                            .rearrange("p (k m) -> p k m", m=WD),
                        in0=dcols.unsqueeze(2).to_broadcast([C, take, WD]),
                        in1=iota[:].unsqueeze(1).to_broadcast([C, take, WD]),
                        op=mybir.AluOpType.is_equal)
                    ohtiles[g0] = oht
                    if len(ohtiles) > 2:
                        ohtiles.pop(next(iter(ohtiles)))
                return oht, gidx - g0

            # ---- layer-1 edge-aligned x stream (fp8) ----
            xetiles = {}

            def ensure_xe(s):
                g0 = (s // XE_SLAB) * XE_SLAB
                xt_ = xetiles.get(g0)
                if xt_ is None:
                    take = min(XE_SLAB, TCH - g0)
                    xt_ = xep.tile([C, XE_SLAB * C], fp8, tag="xe",
                                   name="xebuf")
                    nc.sync.dma_start(out=xt_[:, 0:take * C],
                                      in_=xe_in[:, g0 * C:(g0 + take) * C])
                    xetiles[g0] = xt_
                    if len(xetiles) > 2:
                        xetiles.pop(next(iter(xetiles)))
                return xt_, s - g0

            # ===== layer 1: stream + aggregate + fused dense, per 2-window
            # pair; flush x1 rows per 4 pairs into the segment AG inputs =====
            stage2 = {"t": None, "p0": 0}

            def flush2(pend):
                """Write stage pairs [p0, pend) to their ag_in segment."""
                if stage2["t"] is None:
                    return
                p0 = stage2["p0"]
                npair = pend - p0
                # all pairs of one flush group lie in one segment by
                # construction (seg boundaries are multiples of 4 pairs
                # except the tail, handled by flushing at boundaries)
                s = 0
                while p0 >= sum(SEG_PAIRS[:s + 1]):
                    s += 1
                row0 = p0 * 128 - SEG_START[s]
                nrow = min(npair * 128, SEG_ROWS[s] - row0)
                nfull = nrow // 128
                if nfull > 0:
                    nc.sync.dma_start(
                        out=ag_in[s][row0:row0 + nfull * 128, :]
                            .rearrange("(k p) c -> p k c", p=128),
                        in_=stage2["t"][:, 0:nfull * C]
                            .rearrange("p (k c) -> p k c", c=C))
                if nfull * 128 < nrow:
                    rem = nrow - nfull * 128
                    nc.sync.dma_start(
                        out=ag_in[s][row0 + nfull * 128:row0 + nrow, :],
                        in_=stage2["t"][0:rem, nfull * C:(nfull + 1) * C])
                stage2["t"] = None

            for pair in range(NPAIR):
                zt = hxp.tile([C, C], bf16, tag="z")
                for wi in range(2):
                    w = 2 * pair + wi
                    if w >= W:
                        continue
                    cw = f_chunks[w]
                    psA = psw.tile([C, WD], f32, space="PSUM", tag="pw")
                    for k in range(cw):
                        sidx = win_f_base[w] + k
                        xe_t, xoff = ensure_xe(sidx)
                        oht, ooff = ensure_oh(sidx)
                        nc.tensor.matmul(
                            out=psA[:],
                            lhsT=xe_t[:, xoff * C:(xoff + 1) * C],
                            rhs=oht[:, ooff * WD:(ooff + 1) * WD],
                            start=(k == 0), stop=(k == cw - 1))
                    cols = slice(w * WD, (w + 1) * WD)
                    zc = slice(wi * WD, (wi + 1) * WD)
                    nc.vector.tensor_tensor(out=zt[:, zc], in0=psA[:],
                                            in1=dinvrow[:, cols], op=Mult)
                    nc.vector.tensor_add(out=zt[:, zc], in0=zt[:, zc],
                                         in1=xTl2[:, cols])
                ps2 = psd.tile([C, C], f32, space="PSUM", tag="pd")
                nc.tensor.matmul(out=ps2[:], lhsT=ones1[:], rhs=bias1[:],
                                 start=True, stop=False)
                nc.tensor.matmul(out=ps2[:], lhsT=zt[:], rhs=w1[:],
                                 start=False, stop=True)
                # table rows: dinv * x1  (staged, flushed per 4 pairs)
                if stage2["t"] is None:
                    stage2["t"] = st2p.tile([C, 4 * C], bf16, tag="st2",
                                            name="st2buf")
                    stage2["p0"] = pair
                j = pair - stage2["p0"]
                nc.scalar.activation(stage2["t"][:, j * C:(j + 1) * C],
                                     ps2[:], Relu,
                                     scale=dinvw[:, pair:pair + 1])
                # x1T2 = dinv^2 * x1^T for the layer-2 self term
                xt2 = hxp.tile([C, C], bf16, tag="xt")
                nc.scalar.activation(xt2[:], ps2[:], Relu,
                                     scale=dinvw2[:, pair:pair + 1])
                pt = pstr.tile([C, C], bf16, space="PSUM", tag="tps")
                nc.tensor.transpose(out=pt[:], in_=xt2[:], identity=identb[:])
                nc.scalar.activation(x1T2[:, pair * C:(pair + 1) * C],
                                     pt[:], Copy)
                seg_end = pair + 1 in (16, 32, NPAIR)
                if j == 3 or seg_end:
                    flush2(pair + 1)

            # ===== layer 2: segment gather passes, GpSimd program order:
            # AG0, 10 seg-0 calls, AG1, rest of seg-0, AG2, seg-1, seg-2 =====
            allgather(ag_in[0], tables[0])

            state = {"tiles": {}, "next": [0] * NSEG, "ci": 0}

            def ensure_chunk(s, cidx):
                calls = seg_calls[s]
                while True:
                    for (s2, st), (gt, nch) in state["tiles"].items():
                        if s2 == s and st <= cidx < st + nch:
                            return gt, cidx - st
                    st, nch, col = calls[state["next"][s]]
                    state["next"][s] += 1
                    gt = gp.tile([C, MAX_CALL_CHUNKS * C], bf16, tag="g",
                                 name="gbuf")
                    ni = nch * 128
                    ci = state["ci"]
                    state["ci"] += 1
                    gi = nc.gpsimd.dma_gather(
                        gt[:, 0:nch * C].rearrange("p (k d) -> p k d", d=C),
                        tables[s][:], idx16[:, col:col + nch * 8],
                        ni, ni, C, single_packet=True, queue_num=ci % NQ)
                    add_dep_helper(gi.ins, lib.ins, False, "needs mlp lib")
                    state["tiles"][(s, st)] = (gt, nch)
                    if len(state["tiles"]) > 8:
                        state["tiles"].pop(next(iter(state["tiles"])))
                    if state["ci"] == AG_PREFIX:
                        allgather(ag_in[1], tables[1])

            ps_pool = psp.tile([C, G], f32, space="PSUM", tag="pool")
            zpair = {"t": None}

            def seg_pass(s):
                for w in range(W):
                    cw = g_chunks[s][w]
                    cols = slice(w * WD, (w + 1) * WD)
                    ps = None
                    if cw > 0:
                        ps = psw.tile([C, WD], f32, space="PSUM", tag="pw")
                        for k in range(cw):
                            gt, off = ensure_chunk(s, win_g_base[s][w] + k)
                            oht, ooff = ensure_oh(seg_colbase[s]
                                                  + win_g_base[s][w] + k)
                            nc.tensor.matmul(
                                out=ps[:],
                                lhsT=gt[:, off * C:(off + 1) * C],
                                rhs=oht[:, ooff * WD:(ooff + 1) * WD],
                                start=(k == 0), stop=(k == cw - 1))
                    if s == 0:
                        if ps is not None:
                            nc.vector.tensor_copy(out=accT[:, cols], in_=ps[:])
                        else:
                            nc.vector.memset(accT[:, cols], 0.0)
                    elif s < NSEG - 1:
                        if ps is not None:
                            nc.vector.tensor_add(out=accT[:, cols],
                                                 in0=accT[:, cols], in1=ps[:])
                    else:
                        # final segment: finish z, dense, relu, pool
                        wi = w & 1
                        if wi == 0:
                            zpair["t"] = hxp.tile([C, C], bf16, tag="z",
                                                  name="zbuf")
                        zb = zpair["t"]
                        zc = slice(wi * WD, (wi + 1) * WD)
                        if ps is not None:
                            nc.vector.tensor_add(out=accT[:, cols],
                                                 in0=accT[:, cols], in1=ps[:])
                        nc.vector.tensor_tensor(out=zb[:, zc],
                                                in0=accT[:, cols],
                                                in1=dinvrow[:, cols], op=Mult)
                        nc.vector.tensor_add(out=zb[:, zc], in0=zb[:, zc],
                                             in1=x1T2[:, cols])
                        if wi == 1 or w == W - 1:
                            pair = w // 2
                            ps2 = psd.tile([C, C], f32, space="PSUM",
                                           tag="pd")
                            nc.tensor.matmul(out=ps2[:], lhsT=ones1[:],
                                             rhs=bias2[:],
                                             start=True, stop=False)
                            nc.tensor.matmul(out=ps2[:], lhsT=zb[:],
                                             rhs=w2[:],
                                             start=False, stop=True)
                            x2t = hxp.tile([C, C], bf16, tag="xt")
                            nc.scalar.activation(x2t[:], ps2[:], Relu)
                            nc.tensor.matmul(
                                out=ps_pool[:], lhsT=x2t[:],
                                rhs=selg[:, pair * G:(pair + 1) * G],
                                start=(pair == 0), stop=(pair == NPAIR - 1))

            seg_pass(0)
            allgather(ag_in[2], tables[2])
            seg_pass(1)
            seg_pass(2)

            # ===== pooled all-reduce + final linear =====
            poolT = res.tile([C, G], f32)
            nc.vector.tensor_copy(out=poolT[:], in_=ps_pool[:])
            nc.sync.dma_start(out=ar_in[:], in_=poolT[:])
            nc.gpsimd.collective_compute(
                "AllReduce", mybir.AluOpType.add, replica_groups=rg,
                ins=[ar_in.opt()], outs=[ar_out.opt()])
            poolS = res.tile([C, G], f32)
            nc.sync.dma_start(out=poolS[:], in_=ar_out[:])
            poolb = res.tile([C, G], bf16)
            nc.vector.tensor_copy(out=poolb[:], in_=poolS[:])
            ps_f = psd.tile([G, C_OUT], f32, space="PSUM", tag="pd")
            nc.tensor.matmul(out=ps_f[:], lhsT=poolb[:], rhs=wlin[:],
                             start=True, stop=True)
            fin = res.tile([G, C_OUT], f32)
            nc.vector.tensor_scalar_mul(fin[:], in0=ps_f[:], scalar1=icnt[:])
            nc.vector.tensor_add(out=fin[:], in0=fin[:], in1=blinb[:])
            nc.sync.dma_start(out=out_t[:], in_=fin[:])

    nc.compile()
    nc.m = get_hw_module(nc.m)
    return nc


def _preprocess(edge_index, batch):
    src = np.asarray(edge_index[0], dtype=np.int64)
    dst = np.asarray(edge_index[1], dtype=np.int64)
    batch = np.asarray(batch, dtype=np.int64)

    deg = np.bincount(dst, minlength=N).astype(np.float64) + 1.0
    dinv = (1.0 / np.sqrt(deg)).astype(np.float32)
    counts = np.bincount(batch, minlength=G).astype(np.float64)
    inv_cnt = (1.0 / np.maximum(counts, 1.0)).astype(np.float32)

    order = np.argsort(dst, kind="stable")
    src_s = src[order]
    dst_s = dst[order]
    core_lo = np.searchsorted(dst_s, np.arange(NCORES) * NLOC)
    core_hi = np.searchsorted(dst_s, (np.arange(NCORES) + 1) * NLOC)

    per_core = []
    f_cnt = np.zeros((NCORES, W), np.int64)
    g_cnt = np.zeros((NSEG, NCORES, W), np.int64)
    for c in range(NCORES):
        s = src_s[core_lo[c]:core_hi[c]]
        d = dst_s[core_lo[c]:core_hi[c]] - c * NLOC
        owner = s // NLOC
        pos = s - owner * NLOC
        seg = np.minimum(pos // 2048, NSEG - 1)
        win = d // WD
        # sort by (win, seg) so layer-2 per-seg slices are contiguous
        wlo = np.searchsorted(win, np.arange(W))
        whi = np.searchsorted(win, np.arange(W) + 1)
        wins = []
        for w in range(W):
            sl = slice(wlo[w], whi[w])
            sw = s[sl]
            dw = d[sl] - w * WD
            segw = seg[sl]
            ow = owner[sl]
            pw = pos[sl]
            segdata = []
            for si in range(NSEG):
                m = segw == si
                rw = ow[m] * SEG_ROWS[si] + (pw[m] - SEG_START[si])
                segdata.append((rw, dw[m]))
                g_cnt[si, c, w] = int(m.sum())
            wins.append((sw, dw, segdata))
            f_cnt[c, w] = len(sw)
        per_core.append(wins)

    f_chunks = [int(-(-f_cnt[:, w].max() // 128)) for w in range(W)]
    win_f_base = np.concatenate([[0], np.cumsum(f_chunks)])[:W].astype(int).tolist()
    TCH = int(sum(f_chunks))

    g_chunks = []
    win_g_base = []
    seg_colbase = []
    seg_calls = []
    colbase = TCH
    idx_col = 0
    for si in range(NSEG):
        ch = [int(-(-g_cnt[si, :, w].max() // 128)) for w in range(W)]
        g_chunks.append(ch)
        win_g_base.append(
            np.concatenate([[0], np.cumsum(ch)])[:W].astype(int).tolist())
        seg_colbase.append(colbase)
        colbase += int(sum(ch))
        calls = []
        done = 0
        total = int(sum(ch))
        while done < total:
            take = min(MAX_CALL_CHUNKS, total - done)
            calls.append((done, take, idx_col))
            idx_col += take * 8
            done += take
        seg_calls.append(calls)
    total_drel_cols = colbase
    total_idxcols = idx_col

    plan = {"f_chunks": f_chunks, "win_f_base": win_f_base, "TCH": TCH,
            "g_chunks": g_chunks, "win_g_base": win_g_base,
            "seg_calls": seg_calls, "seg_colbase": seg_colbase,
            "total_drel_cols": total_drel_cols,
            "total_idxcols": total_idxcols}

    return dinv, inv_cnt, plan, per_core


def _host_arrays(plan, per_core, batch, xs):
    """Per-core device input arrays from the edge plan."""
    f_chunks = plan["f_chunks"]
    win_f_base = plan["win_f_base"]
    g_chunks = plan["g_chunks"]
    win_g_base = plan["win_g_base"]
    TCH = plan["TCH"]
    TC = plan["total_drel_cols"]
    TIC = plan["total_idxcols"]
    seg_colbase = plan["seg_colbase"]

    xe_arrs, idx_arrs, drel_arrs, sel_arrs = [], [], [], []
    xsb = xs.astype(FP8)
    for c in range(NCORES):
        xe_t = np.zeros((128, TCH * C), FP8)
        drel_t = np.full((128, TC), -1.0, np.float32)
        seg_idx = [np.zeros(int(sum(g_chunks[s])) * 128, np.int16)
                   for s in range(NSEG)]
        for w in range(W):
            sw, dw, segdata = per_core[c][w]
            # layer-1 edge-aligned stream
            o = win_f_base[w]
            nr = len(sw)
            nch = f_chunks[w]
            buf = np.zeros((nch * 128, C), FP8)
            buf[:nr] = xsb[sw]
            xe_t[:, o * C:(o + nch) * C] = \
                buf.reshape(nch, 128, C).transpose(1, 0, 2).reshape(128, nch * C)
            fl = np.full(nch * 128, -1.0, np.float32)
            fl[:nr] = dw.astype(np.float32)
            drel_t[:, o:o + nch] = fl.reshape(nch, 128).T
            # layer-2 per-segment
            for s in range(NSEG):
                rw, dws = segdata[s]
                o = win_g_base[s][w]
                nch = g_chunks[s][w]
                if nch == 0:
                    continue
                seg_idx[s][o * 128:o * 128 + len(rw)] = rw.astype(np.int16)
                fl = np.full(nch * 128, -1.0, np.float32)
                fl[:len(dws)] = dws.astype(np.float32)
                drel_t[:, seg_colbase[s] + o:
                       seg_colbase[s] + o + nch] = fl.reshape(nch, 128).T
        idx_t = np.zeros((128, TIC), np.int16)
        for s in range(NSEG):
            for s0, take, col in plan["seg_calls"][s]:
                segarr = seg_idx[s][s0 * 128:(s0 + take) * 128]
                wrap = segarr.reshape(take * 8, 16).T
                idx_t[:, col:col + take * 8] = np.tile(wrap, (8, 1))
        xe_arrs.append(xe_t)
        idx_arrs.append(idx_t)
        drel_arrs.append(drel_t.astype(BF16))
        bc = np.full(NPAD, -1.0, np.float32)
        bc[:NLOC] = batch[c * NLOC:(c + 1) * NLOC].astype(np.float32)
        sel = (bc.reshape(NPAIR, 128).T[:, :, None]
               == np.arange(G, dtype=np.float32)[None, None, :]).astype(BF16)
        sel_arrs.append(np.ascontiguousarray(sel.reshape(128, NPAIR * G)))
    return xe_arrs, idx_arrs, drel_arrs, sel_arrs


def kernel(**inputs):
    from concourse import bass_utils

    x = np.asarray(inputs["x"], dtype=np.float32)
    batch = np.asarray(inputs["batch"], dtype=np.int64)
    dinv, inv_cnt, plan, per_core = _preprocess(
        np.asarray(inputs["edge_index"]), batch)

    key = (tuple(plan["f_chunks"]),
           tuple(tuple(ch) for ch in plan["g_chunks"]))
    if key not in _CACHE:
        _CACHE.clear()
        _CACHE[key] = _build_program(plan)
    nc = _CACHE[key]

    b1r = np.asarray(inputs["b1e"], np.float32).reshape(1, C).astype(BF16)
    b2r = np.asarray(inputs["b2e"], np.float32).reshape(1, C).astype(BF16)
    ones1 = np.ones((1, C), np.float32).astype(BF16)
    blinb = np.tile(np.asarray(inputs["blin"], np.float32), (G, 1))
    identb = np.eye(C, dtype=np.float32).astype(BF16)
    iotab = np.tile(np.arange(WD, dtype=np.float32), (C, 1)).astype(BF16)

    xs = x * dinv[:, None]          # D^{-1/2} X
    xe_arrs, idx_arrs, drel_arrs, sel_arrs = _host_arrays(
        plan, per_core, batch, xs)

    in_maps = []
    for c in range(NCORES):
        lo = c * NLOC
        x2l = np.zeros((C, NPAD), np.float32)
        x2l[:, :NLOC] = (x[lo:lo + NLOC]
                         * (dinv[lo:lo + NLOC] ** 2)[:, None]).T
        dv_flat = np.zeros(NPAD, np.float32)
        dv_flat[:NLOC] = dinv[lo:lo + NLOC]
        dwp = dv_flat.reshape(NPAIR, 128).T.copy()
        in_maps.append({
            "xedge": xe_arrs[c],
            "xTl2": x2l.astype(BF16),
            "dinvrow": np.tile(dv_flat, (C, 1)).astype(BF16),
            "idx16": idx_arrs[c], "drelb": drel_arrs[c],
            "selg": sel_arrs[c],
            "identb": identb, "iotab": iotab,
            "w1e": np.asarray(inputs["W1e"], np.float32).astype(BF16),
            "w2e": np.asarray(inputs["W2e"], np.float32).astype(BF16),
            "wlin": np.asarray(inputs["Wlin"], np.float32).astype(BF16),
            "b1row": b1r, "b2row": b2r, "ones1": ones1,
            "dinvw": dwp, "dinvw2": dwp * dwp,
            "blinb": blinb, "invcnt": inv_cnt.reshape(G, 1),
        })

    trace = bool(inputs.get("_trace", False))
    last_err = None
    for _attempt in range(3):
        try:
            res = bass_utils.run_bass_kernel_spmd(nc, in_maps,
                                                  core_ids=list(range(NCORES)),
                                                  trace=trace)
            kernel._last = res
            return np.asarray(res.results[0]["out"], dtype=np.float32)
        except Exception as e:  # transient device-state failures: retry
            last_err = e
    raise last_err


if __name__ == "__main__":
    import jax
    jax.config.update("jax_platforms", "cpu")
    import reference
    inputs = {k: np.asarray(v) for k, v in reference.setup_inputs().items()}
    out = kernel(**inputs)
    exp = np.asarray(reference.reference(**{k: np.asarray(v) for k, v in
                                            reference.setup_inputs().items()}))
    err = np.abs(out - exp).max() / np.abs(exp).max()
    print("rel err:", err)


# revision 14
# speedup vs baseline: 1.1258x; 1.1258x over previous
"""Trainium2 Bass kernel for the DiffPool-style GCN forward pass.

Computation (dead softmax/pool branches of the reference are skipped):
    x1 = relu(Dh (A+I) Dh (x @ W1e) + b1e)
    x2 = relu(Dh (A+I) Dh (x1 @ W2e) + b2e)
    out = (graph_mean_pool(x2) @ Wlin) + blin          -> [64, 10] fp32

Key reassociation: aggregation is linear, so
    x_l+1 = relu([dinv_dst * agg(dinv_src * x_l) + dinv_dst^2 * x_l] @ W + b)

v2 structure (vs the 2-phase v1): the layer-2 SWDGE gather pipeline is the
critical resource (~2.3us per 1024-row call on both Q7 desc-gen and SDMA
drain; >1024-row calls crash the ucode). So the table is split into S=3
source segments, each AllGathered as soon as layer 1 finishes its windows,
and the gather calls for segment s run as one continuous GpSimd stream
starting ~40us into the kernel, overlapping the rest of layer 1, the later
AGs, and all PE/DVE work. Windows are 64 dst wide (halves PE matmul cycles
and one-hot build columns); layer-1's edge-aligned x stream is fp8.
"""

import numpy as np
import ml_dtypes

N = 50000
E = 800000
G = 64
C = 128
C_OUT = 10
NCORES = 8
NLOC = N // NCORES          # 6250
WD = 64                     # dst window width
W = (NLOC + WD - 1) // WD   # 98 windows
NPAIR = (W + 1) // 2        # 49 pairs of windows (128 dst each)
NPAD = W * WD               # 6272
NSEG = 3
SEG_PAIRS = [16, 16, 17]    # window-pairs per source segment
SEG_ROWS = [p * 128 for p in SEG_PAIRS[:-1]] + [NLOC - 32 * 128]  # 2048,2048,2154
SEG_START = [0, 2048, 4096]
MAX_CALL_CHUNKS = 8         # 1024 rows per dma_gather call (ucode cap)
XE_SLAB = 32                # edge-aligned x chunks per stream DMA
OH_GROUP = 32               # one-hot chunks per DVE build
NQ = 4                      # SWDGE queues (ucode max)
AG_PREFIX = 6               # seg-0 gather calls issued before the AG1 trigger

BF16 = ml_dtypes.bfloat16
FP8 = ml_dtypes.float8_e4m3fn

_CACHE = {}


def _build_program(plan):
    import concourse.bacc as bacc
    import concourse.mybir as mybir
    import concourse.tile as tile
    from concourse import library_config
    from concourse.bass_interp import get_hw_module
    from concourse.tile_rust import add_dep_helper

    f32 = mybir.dt.float32
    bf16 = mybir.dt.bfloat16
    fp8 = mybir.dt.float8e4
    i16 = mybir.dt.int16
    Relu = mybir.ActivationFunctionType.Relu
    Copy = mybir.ActivationFunctionType.Copy
    Mult = mybir.AluOpType.mult

    f_chunks = plan["f_chunks"]        # [W] layer-1 chunks per window
    win_f_base = plan["win_f_base"]
    TCH = plan["TCH"]
    g_chunks = plan["g_chunks"]        # [NSEG][W]
    win_g_base = plan["win_g_base"]    # [NSEG][W]
    seg_calls = plan["seg_calls"]      # [NSEG] list of (start_chunk, nch, idx_col)
    seg_colbase = plan["seg_colbase"]  # [NSEG] drel col base
    TC = plan["total_drel_cols"]
    TIC = plan["total_idxcols"]

    nc = bacc.Bacc("TRN2", target_bir_lowering=False, debug=False,
                   num_devices=NCORES, num_swdge_queues=NQ)

    # ---- I/O ----
    xe_in = nc.dram_tensor("xedge", [C, TCH * C], fp8, kind="ExternalInput")
    xtl1_in = nc.dram_tensor("xTl1", [C, NPAD], bf16, kind="ExternalInput")
    dvr_in = nc.dram_tensor("dinvrow", [C, NPAD], bf16, kind="ExternalInput")
    idx_in = nc.dram_tensor("idx16", [C, TIC], i16, kind="ExternalInput")
    drel_in = nc.dram_tensor("drelb", [C, TC], bf16, kind="ExternalInput")
    iota_in = nc.dram_tensor("iotab", [C, WD], bf16, kind="ExternalInput")
    sel_in = nc.dram_tensor("selg", [C, NPAIR * G], bf16, kind="ExternalInput")
    ident_in = nc.dram_tensor("identb", [C, C], bf16, kind="ExternalInput")
    w1_in = nc.dram_tensor("w1e", [C, C], bf16, kind="ExternalInput")
    w2_in = nc.dram_tensor("w2e", [C, C], bf16, kind="ExternalInput")
    wlin_in = nc.dram_tensor("wlin", [C, C_OUT], bf16, kind="ExternalInput")
    b1_in = nc.dram_tensor("b1row", [1, C], bf16, kind="ExternalInput")
    b2_in = nc.dram_tensor("b2row", [1, C], bf16, kind="ExternalInput")
    ones_in = nc.dram_tensor("ones1", [1, C], bf16, kind="ExternalInput")
    dinvw_in = nc.dram_tensor("dinvw", [C, NPAIR], f32, kind="ExternalInput")
    blin_in = nc.dram_tensor("blinb", [G, C_OUT], f32, kind="ExternalInput")
    icnt_in = nc.dram_tensor("invcnt", [G, 1], f32, kind="ExternalInput")
    out_t = nc.dram_tensor("out", [G, C_OUT], f32, kind="ExternalOutput")

    with tile.TileContext(nc) as tc:
        with tc.tile_pool(name="res", bufs=1) as res, \
             tc.tile_pool(name="gp", bufs=8) as gp, \
             tc.tile_pool(name="xep", bufs=3) as xep, \
             tc.tile_pool(name="ohp", bufs=3) as ohp, \
             tc.tile_pool(name="st2", bufs=3) as st2p, \
             tc.tile_pool(name="hx", bufs=4) as hxp, \
             tc.tile_pool(name="psw", bufs=2, space="PSUM") as psw, \
             tc.tile_pool(name="psd", bufs=2, space="PSUM") as psd, \
             tc.tile_pool(name="pstr", bufs=1, space="PSUM") as pstr, \
             tc.tile_pool(name="psp", bufs=1, space="PSUM") as psp, \
             tc.tile_pool(name="dram", bufs=1, space="DRAM") as dram:

            lib = nc.gpsimd.load_library(library_config.mlp)

            def load_res(name, src, shape, dt=f32):
                t = res.tile(shape, dt, tag=name)
                nc.sync.dma_start(out=t[:], in_=src[:])
                return t

            idx16 = load_res("r_idx", idx_in, [C, TIC], i16)
            drel = load_res("r_drel", drel_in, [C, TC], bf16)
            iota = load_res("r_iota", iota_in, [C, WD], bf16)
            xTl1 = load_res("r_xtl1", xtl1_in, [C, NPAD], bf16)
            dinvrow = load_res("r_dvr", dvr_in, [C, NPAD], bf16)
            selg = load_res("r_sel", sel_in, [C, NPAIR * G], bf16)
            identb = load_res("r_id", ident_in, [C, C], bf16)
            w1 = load_res("r_w1", w1_in, [C, C], bf16)
            w2 = load_res("r_w2", w2_in, [C, C], bf16)
            wlin = load_res("r_wl", wlin_in, [C, C_OUT], bf16)
            bias1 = load_res("r_b1", b1_in, [1, C], bf16)
            bias2 = load_res("r_b2", b2_in, [1, C], bf16)
            ones1 = load_res("r_on", ones_in, [1, C], bf16)
            dinvw = load_res("r_dw", dinvw_in, [C, NPAIR])
            blinb = load_res("r_bl", blin_in, [G, C_OUT])
            icnt = load_res("r_ic", icnt_in, [G, 1])

            x1T1 = res.tile([C, NPAD], bf16)   # dinv * x1^T (self seed)
            accT = res.tile([C, NPAD], bf16)   # layer-2 partial agg + self

            # ---- DRAM buffers ----
            ag_in = [dram.tile([SEG_ROWS[s], C], bf16, name=f"agin{s}")
                     for s in range(NSEG)]
            tables = [dram.tile([SEG_ROWS[s] * NCORES, C], bf16,
                                name=f"table{s}", addr_space="Shared")
                      for s in range(NSEG)]
            ar_in = dram.tile([C, G], f32)
            ar_out = dram.tile([C, G], f32)
            rg = [list(range(NCORES))]

            def allgather(src, dst):
                nc.gpsimd.collective_compute(
                    "AllGather", mybir.AluOpType.bypass, replica_groups=rg,
                    ins=[src.opt()], outs=[dst.opt()])

            # ---- shared one-hot builder over unified drel col space ----
            ohtiles = {}

            def ensure_oh(gidx):
                g0 = (gidx // OH_GROUP) * OH_GROUP
                oht = ohtiles.get(g0)
                if oht is None:
                    take = min(OH_GROUP, TC - g0)
                    oht = ohp.tile([C, OH_GROUP * WD], bf16, tag="oh",
                                   name="ohbuf")
                    dcols = drel[:, g0:g0 + take]
                    nc.vector.tensor_tensor(
                        out=oht[:, 0:take * WD]
                            .rearrange("p (k m) -> p k m", m=WD),
                        in0=dcols.unsqueeze(2).to_broadcast([C, take, WD]),
                        in1=iota[:].unsqueeze(1).to_broadcast([C, take, WD]),
                        op=mybir.AluOpType.is_equal)
                    ohtiles[g0] = oht
                    if len(ohtiles) > 2:
                        ohtiles.pop(next(iter(ohtiles)))
                return oht, gidx - g0

            # ---- layer-1 edge-aligned x stream (fp8) ----
            xetiles = {}

            def ensure_xe(s):
                g0 = (s // XE_SLAB) * XE_SLAB
                xt_ = xetiles.get(g0)
                if xt_ is None:
                    take = min(XE_SLAB, TCH - g0)
                    xt_ = xep.tile([C, XE_SLAB * C], fp8, tag="xe",
                                   name="xebuf")
                    nc.sync.dma_start(out=xt_[:, 0:take * C],
                                      in_=xe_in[:, g0 * C:(g0 + take) * C])
                    xetiles[g0] = xt_
                    if len(xetiles) > 2:
                        xetiles.pop(next(iter(xetiles)))
                return xt_, s - g0

            # ===== layer 1: stream + aggregate + fused dense, per 2-window
            # pair; flush x1 rows per 4 pairs into the segment AG inputs =====
            stage2 = {"t": None, "p0": 0}

            def flush2(pend):
                """Write stage pairs [p0, pend) to their ag_in segment."""
                if stage2["t"] is None:
                    return
                p0 = stage2["p0"]
                npair = pend - p0
                # all pairs of one flush group lie in one segment by
                # construction (seg boundaries are multiples of 4 pairs
                # except the tail, handled by flushing at boundaries)
                s = 0
                while p0 >= sum(SEG_PAIRS[:s + 1]):
                    s += 1
                row0 = p0 * 128 - SEG_START[s]
                nrow = min(npair * 128, SEG_ROWS[s] - row0)
                nfull = nrow // 128
                if nfull > 0:
                    nc.sync.dma_start(
                        out=ag_in[s][row0:row0 + nfull * 128, :]
                            .rearrange("(k p) c -> p k c", p=128),
                        in_=stage2["t"][:, 0:nfull * C]
                            .rearrange("p (k c) -> p k c", c=C))
                if nfull * 128 < nrow:
                    rem = nrow - nfull * 128
                    nc.sync.dma_start(
                        out=ag_in[s][row0 + nfull * 128:row0 + nrow, :],
                        in_=stage2["t"][0:rem, nfull * C:(nfull + 1) * C])
                stage2["t"] = None

            for pair in range(NPAIR):
                zt = hxp.tile([C, C], bf16, tag="z")
                for wi in range(2):
                    w = 2 * pair + wi
                    if w >= W:
                        continue
                    cw = f_chunks[w]
                    cols = slice(w * WD, (w + 1) * WD)
                    psA = psw.tile([C, WD], f32, space="PSUM", tag="pw")
                    nc.tensor.matmul(out=psA[:], lhsT=identb[:],
                                     rhs=xTl1[:, cols],
                                     start=True, stop=(cw == 0))
                    for k in range(cw):
                        sidx = win_f_base[w] + k
                        xe_t, xoff = ensure_xe(sidx)
                        oht, ooff = ensure_oh(sidx)
                        nc.tensor.matmul(
                            out=psA[:],
                            lhsT=xe_t[:, xoff * C:(xoff + 1) * C],
                            rhs=oht[:, ooff * WD:(ooff + 1) * WD],
                            start=False, stop=(k == cw - 1))
                    zc = slice(wi * WD, (wi + 1) * WD)
                    nc.vector.tensor_tensor(out=zt[:, zc], in0=psA[:],
                                            in1=dinvrow[:, cols], op=Mult)
                ps2 = psd.tile([C, C], f32, space="PSUM", tag="pd")
                nc.tensor.matmul(out=ps2[:], lhsT=ones1[:], rhs=bias1[:],
                                 start=True, stop=False)
                nc.tensor.matmul(out=ps2[:], lhsT=zt[:], rhs=w1[:],
                                 start=False, stop=True)
                # table rows: dinv * x1  (staged, flushed per 4 pairs)
                if stage2["t"] is None:
                    stage2["t"] = st2p.tile([C, 4 * C], bf16, tag="st2",
                                            name="st2buf")
                    stage2["p0"] = pair
                j = pair - stage2["p0"]
                nc.scalar.activation(stage2["t"][:, j * C:(j + 1) * C],
                                     ps2[:], Relu,
                                     scale=dinvw[:, pair:pair + 1])
                # x1T1 = (dinv * x1)^T: transpose of the stage rows
                pt = pstr.tile([C, C], bf16, space="PSUM", tag="tps")
                nc.tensor.transpose(out=pt[:],
                                    in_=stage2["t"][:, j * C:(j + 1) * C],
                                    identity=identb[:])
                nc.scalar.activation(x1T1[:, pair * C:(pair + 1) * C],
                                     pt[:], Copy)
                seg_end = pair + 1 in (16, 32, NPAIR)
                if j == 3 or seg_end:
                    flush2(pair + 1)

            # ===== layer 2: segment gather passes, GpSimd program order:
            # AG0, 10 seg-0 calls, AG1, rest of seg-0, AG2, seg-1, seg-2 =====
            allgather(ag_in[0], tables[0])

            state = {"tiles": {}, "next": [0] * NSEG, "ci": 0}

            def ensure_chunk(s, cidx):
                calls = seg_calls[s]
                while True:
                    for (s2, st), (gt, nch) in state["tiles"].items():
                        if s2 == s and st <= cidx < st + nch:
                            return gt, cidx - st
                    st, nch, col = calls[state["next"][s]]
                    state["next"][s] += 1
                    gt = gp.tile([C, MAX_CALL_CHUNKS * C], bf16, tag="g",
                                 name="gbuf")
                    ni = nch * 128
                    ci = state["ci"]
                    state["ci"] += 1
                    gi = nc.gpsimd.dma_gather(
                        gt[:, 0:nch * C].rearrange("p (k d) -> p k d", d=C),
                        tables[s][:], idx16[:, col:col + nch * 8],
                        ni, ni, C, single_packet=True, queue_num=ci % NQ)
                    add_dep_helper(gi.ins, lib.ins, False, "needs mlp lib")
                    state["tiles"][(s, st)] = (gt, nch)
                    if len(state["tiles"]) > 8:
                        state["tiles"].pop(next(iter(state["tiles"])))
                    if state["ci"] == AG_PREFIX:
                        allgather(ag_in[1], tables[1])

            ps_pool = psp.tile([C, G], f32, space="PSUM", tag="pool")
            zpair = {"t": None}

            def seg_pass(s):
                for w in range(W):
                    cw = g_chunks[s][w]
                    cols = slice(w * WD, (w + 1) * WD)
                    seed = x1T1 if s == 0 else accT
                    ps = psw.tile([C, WD], f32, space="PSUM", tag="pw")
                    nc.tensor.matmul(out=ps[:], lhsT=identb[:],
                                     rhs=seed[:, cols],
                                     start=True, stop=(cw == 0))
                    for k in range(cw):
                        gt, off = ensure_chunk(s, win_g_base[s][w] + k)
                        oht, ooff = ensure_oh(seg_colbase[s]
                                              + win_g_base[s][w] + k)
                        nc.tensor.matmul(
                            out=ps[:],
                            lhsT=gt[:, off * C:(off + 1) * C],
                            rhs=oht[:, ooff * WD:(ooff + 1) * WD],
                            start=False, stop=(k == cw - 1))
                    if s < NSEG - 1:
                        nc.scalar.activation(accT[:, cols], ps[:], Copy)
                    else:
                        # final segment: finish z, dense, relu, pool
                        wi = w & 1
                        if wi == 0:
                            zpair["t"] = hxp.tile([C, C], bf16, tag="z",
                                                  name="zbuf")
                        zb = zpair["t"]
                        zc = slice(wi * WD, (wi + 1) * WD)
                        nc.vector.tensor_tensor(out=zb[:, zc],
                                                in0=ps[:],
                                                in1=dinvrow[:, cols], op=Mult)
                        if wi == 1 or w == W - 1:
                            pair = w // 2
                            ps2 = psd.tile([C, C], f32, space="PSUM",
                                           tag="pd")
                            nc.tensor.matmul(out=ps2[:], lhsT=ones1[:],
                                             rhs=bias2[:],
                                             start=True, stop=False)
                            nc.tensor.matmul(out=ps2[:], lhsT=zb[:],
                                             rhs=w2[:],
                                             start=False, stop=True)
                            x2t = hxp.tile([C, C], bf16, tag="xt")
                            nc.scalar.activation(x2t[:], ps2[:], Relu)
                            nc.tensor.matmul(
                                out=ps_pool[:], lhsT=x2t[:],
                                rhs=selg[:, pair * G:(pair + 1) * G],
                                start=(pair == 0), stop=(pair == NPAIR - 1))

            seg_pass(0)
            allgather(ag_in[2], tables[2])
            seg_pass(1)
            seg_pass(2)

            # ===== pooled all-reduce + final linear =====
            poolT = res.tile([C, G], f32)
            nc.vector.tensor_copy(out=poolT[:], in_=ps_pool[:])
            nc.sync.dma_start(out=ar_in[:], in_=poolT[:])
            nc.gpsimd.collective_compute(
                "AllReduce", mybir.AluOpType.add, replica_groups=rg,
                ins=[ar_in.opt()], outs=[ar_out.opt()])
            poolS = res.tile([C, G], f32)
            nc.sync.dma_start(out=poolS[:], in_=ar_out[:])
            poolb = res.tile([C, G], bf16)
            nc.vector.tensor_copy(out=poolb[:], in_=poolS[:])
            ps_f = psd.tile([G, C_OUT], f32, space="PSUM", tag="pd")
            nc.tensor.matmul(out=ps_f[:], lhsT=poolb[:], rhs=wlin[:],
                             start=True, stop=True)
            fin = res.tile([G, C_OUT], f32)
            nc.vector.tensor_scalar_mul(fin[:], in0=ps_f[:], scalar1=icnt[:])
            nc.vector.tensor_add(out=fin[:], in0=fin[:], in1=blinb[:])
            nc.sync.dma_start(out=out_t[:], in_=fin[:])

    nc.compile()
    nc.m = get_hw_module(nc.m)
    return nc


def _preprocess(edge_index, batch):
    src = np.asarray(edge_index[0], dtype=np.int64)
    dst = np.asarray(edge_index[1], dtype=np.int64)
    batch = np.asarray(batch, dtype=np.int64)

    deg = np.bincount(dst, minlength=N).astype(np.float64) + 1.0
    dinv = (1.0 / np.sqrt(deg)).astype(np.float32)
    counts = np.bincount(batch, minlength=G).astype(np.float64)
    inv_cnt = (1.0 / np.maximum(counts, 1.0)).astype(np.float32)

    order = np.argsort(dst, kind="stable")
    src_s = src[order]
    dst_s = dst[order]
    core_lo = np.searchsorted(dst_s, np.arange(NCORES) * NLOC)
    core_hi = np.searchsorted(dst_s, (np.arange(NCORES) + 1) * NLOC)

    per_core = []
    f_cnt = np.zeros((NCORES, W), np.int64)
    g_cnt = np.zeros((NSEG, NCORES, W), np.int64)
    for c in range(NCORES):
        s = src_s[core_lo[c]:core_hi[c]]
        d = dst_s[core_lo[c]:core_hi[c]] - c * NLOC
        owner = s // NLOC
        pos = s - owner * NLOC
        seg = np.minimum(pos // 2048, NSEG - 1)
        win = d // WD
        # sort by (win, seg) so layer-2 per-seg slices are contiguous
        wlo = np.searchsorted(win, np.arange(W))
        whi = np.searchsorted(win, np.arange(W) + 1)
        wins = []
        for w in range(W):
            sl = slice(wlo[w], whi[w])
            sw = s[sl]
            dw = d[sl] - w * WD
            segw = seg[sl]
            ow = owner[sl]
            pw = pos[sl]
            segdata = []
            for si in range(NSEG):
                m = segw == si
                rw = ow[m] * SEG_ROWS[si] + (pw[m] - SEG_START[si])
                segdata.append((rw, dw[m]))
                g_cnt[si, c, w] = int(m.sum())
            wins.append((sw, dw, segdata))
            f_cnt[c, w] = len(sw)
        per_core.append(wins)

    f_chunks = [int(-(-f_cnt[:, w].max() // 128)) for w in range(W)]
    win_f_base = np.concatenate([[0], np.cumsum(f_chunks)])[:W].astype(int).tolist()
    TCH = int(sum(f_chunks))

    g_chunks = []
    win_g_base = []
    seg_colbase = []
    seg_calls = []
    colbase = TCH
    idx_col = 0
    for si in range(NSEG):
        ch = [int(-(-g_cnt[si, :, w].max() // 128)) for w in range(W)]
        g_chunks.append(ch)
        win_g_base.append(
            np.concatenate([[0], np.cumsum(ch)])[:W].astype(int).tolist())
        seg_colbase.append(colbase)
        colbase += int(sum(ch))
        calls = []
        done = 0
        total = int(sum(ch))
        while done < total:
            take = min(MAX_CALL_CHUNKS, total - done)
            calls.append((done, take, idx_col))
            idx_col += take * 8
            done += take
        seg_calls.append(calls)
    total_drel_cols = colbase
    total_idxcols = idx_col

    plan = {"f_chunks": f_chunks, "win_f_base": win_f_base, "TCH": TCH,
            "g_chunks": g_chunks, "win_g_base": win_g_base,
            "seg_calls": seg_calls, "seg_colbase": seg_colbase,
            "total_drel_cols": total_drel_cols,
            "total_idxcols": total_idxcols}

    return dinv, inv_cnt, plan, per_core


def _host_arrays(plan, per_core, batch, xs):
    """Per-core device input arrays from the edge plan."""
    f_chunks = plan["f_chunks"]
    win_f_base = plan["win_f_base"]
    g_chunks = plan["g_chunks"]
    win_g_base = plan["win_g_base"]
    TCH = plan["TCH"]
    TC = plan["total_drel_cols"]
    TIC = plan["total_idxcols"]
    seg_colbase = plan["seg_colbase"]

    xe_arrs, idx_arrs, drel_arrs, sel_arrs = [], [], [], []
    xsb = xs.astype(FP8)
    for c in range(NCORES):
        xe_t = np.zeros((128, TCH * C), FP8)
        drel_t = np.full((128, TC), -1.0, np.float32)
        seg_idx = [np.zeros(int(sum(g_chunks[s])) * 128, np.int16)
                   for s in range(NSEG)]
        for w in range(W):
            sw, dw, segdata = per_core[c][w]
            # layer-1 edge-aligned stream
            o = win_f_base[w]
            nr = len(sw)
            nch = f_chunks[w]
            buf = np.zeros((nch * 128, C), FP8)
            buf[:nr] = xsb[sw]
            xe_t[:, o * C:(o + nch) * C] = \
                buf.reshape(nch, 128, C).transpose(1, 0, 2).reshape(128, nch * C)
            fl = np.full(nch * 128, -1.0, np.float32)
            fl[:nr] = dw.astype(np.float32)
            drel_t[:, o:o + nch] = fl.reshape(nch, 128).T
            # layer-2 per-segment
            for s in range(NSEG):
                rw, dws = segdata[s]
                o = win_g_base[s][w]
                nch = g_chunks[s][w]
                if nch == 0:
                    continue
                seg_idx[s][o * 128:o * 128 + len(rw)] = rw.astype(np.int16)
                fl = np.full(nch * 128, -1.0, np.float32)
                fl[:len(dws)] = dws.astype(np.float32)
                drel_t[:, seg_colbase[s] + o:
                       seg_colbase[s] + o + nch] = fl.reshape(nch, 128).T
        idx_t = np.zeros((128, TIC), np.int16)
        for s in range(NSEG):
            for s0, take, col in plan["seg_calls"][s]:
                segarr = seg_idx[s][s0 * 128:(s0 + take) * 128]
                wrap = segarr.reshape(take * 8, 16).T
                idx_t[:, col:col + take * 8] = np.tile(wrap, (8, 1))
        xe_arrs.append(xe_t)
        idx_arrs.append(idx_t)
        drel_arrs.append(drel_t.astype(BF16))
        bc = np.full(NPAD, -1.0, np.float32)
        bc[:NLOC] = batch[c * NLOC:(c + 1) * NLOC].astype(np.float32)
        sel = (bc.reshape(NPAIR, 128).T[:, :, None]
               == np.arange(G, dtype=np.float32)[None, None, :]).astype(BF16)
        sel_arrs.append(np.ascontiguousarray(sel.reshape(128, NPAIR * G)))
    return xe_arrs, idx_arrs, drel_arrs, sel_arrs


def kernel(**inputs):
    from concourse import bass_utils

    x = np.asarray(inputs["x"], dtype=np.float32)
    batch = np.asarray(inputs["batch"], dtype=np.int64)
    dinv, inv_cnt, plan, per_core = _preprocess(
        np.asarray(inputs["edge_index"]), batch)

    key = (tuple(plan["f_chunks"]),
           tuple(tuple(ch) for ch in plan["g_chunks"]))
    if key not in _CACHE:
        _CACHE.clear()
        _CACHE[key] = _build_program(plan)
    nc = _CACHE[key]

    b1r = np.asarray(inputs["b1e"], np.float32).reshape(1, C).astype(BF16)
    b2r = np.asarray(inputs["b2e"], np.float32).reshape(1, C).astype(BF16)
    ones1 = np.ones((1, C), np.float32).astype(BF16)
    blinb = np.tile(np.asarray(inputs["blin"], np.float32), (G, 1))
    identb = np.eye(C, dtype=np.float32).astype(BF16)
    iotab = np.tile(np.arange(WD, dtype=np.float32), (C, 1)).astype(BF16)

    xs = x * dinv[:, None]          # D^{-1/2} X
    xe_arrs, idx_arrs, drel_arrs, sel_arrs = _host_arrays(
        plan, per_core, batch, xs)

    in_maps = []
    for c in range(NCORES):
        lo = c * NLOC
        x2l = np.zeros((C, NPAD), np.float32)
        x2l[:, :NLOC] = (x[lo:lo + NLOC]
                         * dinv[lo:lo + NLOC][:, None]).T
        dv_flat = np.zeros(NPAD, np.float32)
        dv_flat[:NLOC] = dinv[lo:lo + NLOC]
        dwp = dv_flat.reshape(NPAIR, 128).T.copy()
        in_maps.append({
            "xedge": xe_arrs[c],
            "xTl1": x2l.astype(BF16),
            "dinvrow": np.tile(dv_flat, (C, 1)).astype(BF16),
            "idx16": idx_arrs[c], "drelb": drel_arrs[c],
            "selg": sel_arrs[c],
            "identb": identb, "iotab": iotab,
            "w1e": np.asarray(inputs["W1e"], np.float32).astype(BF16),
            "w2e": np.asarray(inputs["W2e"], np.float32).astype(BF16),
            "wlin": np.asarray(inputs["Wlin"], np.float32).astype(BF16),
            "b1row": b1r, "b2row": b2r, "ones1": ones1,
            "dinvw": dwp,
            "blinb": blinb, "invcnt": inv_cnt.reshape(G, 1),
        })

    trace = bool(inputs.get("_trace", False))
    last_err = None
    for _attempt in range(3):
        try:
            res = bass_utils.run_bass_kernel_spmd(nc, in_maps,
                                                  core_ids=list(range(NCORES)),
                                                  trace=trace)
            kernel._last = res
            return np.asarray(res.results[0]["out"], dtype=np.float32)
        except Exception as e:  # transient device-state failures: retry
            last_err = e
    raise last_err


if __name__ == "__main__":
    import jax
    jax.config.update("jax_platforms", "cpu")
    import reference
    inputs = {k: np.asarray(v) for k, v in reference.setup_inputs().items()}
    out = kernel(**inputs)
    exp = np.asarray(reference.reference(**{k: np.asarray(v) for k, v in
                                            reference.setup_inputs().items()}))
    err = np.abs(out - exp).max() / np.abs(exp).max()
    print("rel err:", err)
